# revision 19
# baseline (speedup 1.0000x reference)
"""MoE kernel for Trainium2 (8 NeuronCores, expert-parallel).

Strategy:
  - Router (tiny: [8192,1024]@[1024,8]) + top-2 selection + load-balance loss
    run on host with jax-CPU, bit-matching the reference implementation.
  - Tokens are gathered per expert on host ("all-to-all dispatch" done at
    input-sharding time) and shipped pre-transposed as xT=[D, cap] so the
    expert weights (Wgu [D,2H], Wd [H,D]) are used directly as matmul lhsT
    operands -- zero on-device transposes.
  - Each of the 8 cores runs the dense FFN for one expert in fp16 inputs /
    fp32 accumulate: G=x@Wg, U=x@Wu, A=silu(G)*U, Y=A@Wd.
  - Host applies the top-2 combine weights while scatter-adding each
    expert's rows back into the full [8192,1024] output ("combine").
  - alpha != 1 fallback: the same compiled kernel is run a second time with
    Wx in place of Wg, and the two outputs are blended per expert
    (alpha*Y1 + (1-alpha)*Y2) before the combine -- exact by linearity of
    the down-projection.
"""

import numpy as np

B, S, D = 4, 2048, 1024
E, TOPK, H = 8, 2, 1408
T = B * S
P = 128
CH_OPTIONS = (512, 448, 384, 320, 256)  # token chunk (matmul moving free dim)
KD, KH = D // P, H // P     # k-tiles over D, tiles over H
NCORES = 8
LB_WEIGHT = 0.01

_BUILD_CACHE = {}
LAST_RESULT = None          # test.py introspection (exec_time_ns etc.)


def _pick_geometry(max_load):
    """Pick (cap, CH) minimizing modeled PE stream time."""
    best = None
    for ch in CH_OPTIONS:
        nchunk = max(1, -(-max_load // ch))
        cost = nchunk * (2 * KH * KD + KD * KH) * (ch / 2.4 + 2.5)
        if best is None or cost < best[0]:
            best = (cost, nchunk * ch, ch)
    return best[1], best[2]


def _build_ffn(cap, CH):
    """One-expert dense FFN, transposed layout. All cores run this program."""
    import concourse.bacc as bacc
    import concourse.mybir as mybir
    import concourse.tile as tile

    f32 = mybir.dt.float32
    f16 = mybir.dt.float16
    nchunk = cap // CH

    nc = bacc.Bacc(None, target_bir_lowering=False)
    xT = nc.dram_tensor("xT", [D, cap], f16, kind="ExternalInput")
    wgu = nc.dram_tensor("wgu", [D, 2 * H], f16, kind="ExternalInput")
    wd = nc.dram_tensor("wd", [H, D], f16, kind="ExternalInput")
    yT = nc.dram_tensor("yT", [D, cap], f32, kind="ExternalOutput")

    with tile.TileContext(nc) as tc:
        with (
            tc.tile_pool(name="wpool", bufs=1) as wpool,
            tc.tile_pool(name="xpool", bufs=2) as xpool,
            tc.tile_pool(name="apool", bufs=1) as apool,
            tc.tile_pool(name="spool", bufs=2) as spool,
            tc.tile_pool(name="ypool", bufs=3) as ypool,
            tc.tile_pool(name="psgu", bufs=6, space="PSUM") as psgu,
            tc.tile_pool(name="psy", bufs=2, space="PSUM") as psy,
        ):
            wgu_t = wgu.rearrange("(ko p) h -> p ko h", p=P)
            wd_t = wd.rearrange("(ko p) d -> p ko d", p=P)
            xT_t = xT.rearrange("(ko p) n -> p ko n", p=P)

            def load_x(c):
                # chunk 0: per-k DMAs so the very first matmul only waits for
                # x[k=0] + wgu[0]; later chunks: one DMA, prefetched under
                # plenty of compute
                if c == 0:
                    xs = []
                    for k in range(KD):
                        t = xpool.tile([P, CH], f16, tag=f"x0k{k}", name=f"x0k{k}")
                        nc.sync.dma_start(t[:], xT_t[:, k, 0:CH])
                        xs.append(t)
                    return xs
                t = xpool.tile([P, KD, CH], f16, tag="x", name=f"xc{c}")
                nc.sync.dma_start(t[:], xT_t[:, :, c * CH:(c + 1) * CH])
                return [t[:, k, :] for k in range(KD)]

            x_sb = []
            wgu_sb = []
            for k in range(KD):
                tx = xpool.tile([P, CH], f16, tag=f"x0k{k}", name=f"x0k{k}")
                nc.sync.dma_start(tx[:], xT_t[:, k, 0:CH])
                x_sb.append(tx)
                tg = wpool.tile([P, 2 * H], f16, tag=f"wgu{k}", name=f"wgu{k}")
                nc.sync.dma_start(tg[:], wgu_t[:, k, :])
                wgu_sb.append(tg)
            wd_sb = []
            for k in range(KH):
                td = wpool.tile([P, D], f16, tag=f"wd{k}", name=f"wd{k}")
                nc.sync.dma_start(td[:], wd_t[:, k, :])
                wd_sb.append(td)

            for c in range(nchunk):
                if c > 0:
                    x_sb = load_x(c)
                a_sb = [
                    apool.tile([P, CH], f16, tag=f"a{h}", name=f"a{h}")
                    for h in range(KH)
                ]
                for h in range(KH):
                    g_ps = psgu.tile([P, CH], f32, tag="gu")
                    u_ps = psgu.tile([P, CH], f32, tag="gu")
                    gs = slice(h * P, (h + 1) * P)
                    us = slice(H + h * P, H + (h + 1) * P)
                    for k in range(KD):
                        nc.tensor.matmul(
                            g_ps[:], wgu_sb[k][:, gs], x_sb[k],
                            start=(k == 0), stop=(k == KD - 1),
                        )
                    for k in range(KD):
                        nc.tensor.matmul(
                            u_ps[:], wgu_sb[k][:, us], x_sb[k],
                            start=(k == 0), stop=(k == KD - 1),
                        )
                    s_sb = spool.tile([P, CH], f32, tag="s")
                    nc.scalar.activation(
                        s_sb[:], g_ps[:], mybir.ActivationFunctionType.Silu
                    )
                    nc.vector.tensor_mul(a_sb[h][:], s_sb[:], u_ps[:])

                for m in range(KD):
                    y_ps = psy.tile([P, CH], f32, tag="y")
                    ms = slice(m * P, (m + 1) * P)
                    for h in range(KH):
                        nc.tensor.matmul(
                            y_ps[:], wd_sb[h][:, ms], a_sb[h][:],
                            start=(h == 0), stop=(h == KH - 1),
                        )
                    y_sb = ypool.tile([P, CH], f32, tag="y")
                    nc.vector.tensor_copy(y_sb[:], y_ps[:])
                    nc.sync.dma_start(yT[ms, c * CH:(c + 1) * CH], y_sb[:])

    nc.compile()
    return nc


def _route(xf, Wr):
    """Top-2 routing + lb loss, bit-matching the reference (jax on CPU)."""
    import jax
    import jax.numpy as jnp

    cpu = jax.devices("cpu")[0]
    with jax.default_device(cpu):
        probs = jax.nn.softmax(jnp.asarray(xf) @ jnp.asarray(Wr), axis=-1)
        topk_p, topk_i = jax.lax.top_k(probs, TOPK)
        topk_p = topk_p / jnp.sum(topk_p, axis=-1, keepdims=True)
        usage = jnp.mean(probs, axis=0)
        lb = LB_WEIGHT * jnp.mean((usage - 1.0 / E) ** 2)
        return (
            np.asarray(topk_p),
            np.asarray(topk_i),
            np.asarray(lb, dtype=np.float32),
        )


def _run_spmd(nc, in_maps):
    global LAST_RESULT
    from concourse.bass_utils import run_bass_kernel_spmd

    res = run_bass_kernel_spmd(nc, in_maps, list(range(NCORES)))
    LAST_RESULT = res
    return res


def kernel(x, Wr, Wg, Wu, Wx, Wd, alpha):
    x = np.asarray(x, dtype=np.float32)
    Wr = np.asarray(Wr, dtype=np.float32)
    Wg = np.asarray(Wg, dtype=np.float32)
    Wu = np.asarray(Wu, dtype=np.float32)
    Wx = np.asarray(Wx, dtype=np.float32)
    Wd = np.asarray(Wd, dtype=np.float32)
    alpha = np.asarray(alpha, dtype=np.float32)

    xf = x.reshape(T, D)
    topk_p, topk_i, lb = _route(xf, Wr)

    # token ids + combine weight per expert
    ids = []
    wts = []
    for e in range(E):
        sel = (topk_i == e)
        tok = np.nonzero(sel.any(axis=1))[0]
        ids.append(tok)
        wts.append(np.where(sel[tok, 0], topk_p[tok, 0], topk_p[tok, 1]))
    cap, ch = _pick_geometry(max(len(t) for t in ids))

    if (cap, ch) not in _BUILD_CACHE:
        _BUILD_CACHE[(cap, ch)] = _build_ffn(cap, ch)
    nc = _BUILD_CACHE[(cap, ch)]

    xT = np.zeros((E, D, cap), dtype=np.float16)
    for e in range(E):
        xT[e, :, : len(ids[e])] = xf[ids[e]].T.astype(np.float16)

    in_maps = [
        {"xT": xT[e],
         "wgu": np.concatenate([Wg[e], Wu[e]], axis=1).astype(np.float16),
         "wd": Wd[e].astype(np.float16)}
        for e in range(E)
    ]
    res = _run_spmd(nc, in_maps)
    y = [res.results[e]["yT"] for e in range(E)]

    if not np.all(alpha == 1.0):
        in_maps2 = [
            dict(m, wgu=np.concatenate([Wx[e], Wu[e]], axis=1).astype(np.float16))
            for e, m in enumerate(in_maps)
        ]
        res2 = _run_spmd(nc, in_maps2)
        y = [alpha[e] * y[e] + (1.0 - alpha[e]) * res2.results[e]["yT"]
             for e in range(E)]

    out = np.zeros((T, D), dtype=np.float32)
    for e in range(E):
        n = len(ids[e])
        out[ids[e]] += wts[e][:, None] * y[e].T[:n]

    return out.reshape(B, S, D), lb


# revision 20
# speedup vs baseline: 1.0211x; 1.0211x over previous
"""MoE kernel for Trainium2 (8 NeuronCores, expert-parallel).

Strategy:
  - Router (tiny: [8192,1024]@[1024,8]) + top-2 selection + load-balance loss
    run on host with jax-CPU, bit-matching the reference implementation.
  - Tokens are gathered per expert on host ("all-to-all dispatch" done at
    input-sharding time) and shipped pre-transposed as xT=[D, cap] so the
    expert weights (Wgu [D,2H], Wd [H,D]) are used directly as matmul lhsT
    operands -- zero on-device transposes.
  - Each of the 8 cores runs the dense FFN for one expert in fp16 inputs /
    fp32 accumulate: G=x@Wg, U=x@Wu, A=silu(G)*U, Y=A@Wd.
  - Host applies the top-2 combine weights while scatter-adding each
    expert's rows back into the full [8192,1024] output ("combine").
  - alpha != 1 fallback: the same compiled kernel is run a second time with
    Wx in place of Wg, and the two outputs are blended per expert
    (alpha*Y1 + (1-alpha)*Y2) before the combine -- exact by linearity of
    the down-projection.
"""

import numpy as np

B, S, D = 4, 2048, 1024
E, TOPK, H = 8, 2, 1408
T = B * S
P = 128
CH_OPTIONS = (512, 448, 384, 320, 256)  # token chunk (matmul moving free dim)
KD, KH = D // P, H // P     # k-tiles over D, tiles over H
NCORES = 8
LB_WEIGHT = 0.01

_BUILD_CACHE = {}
LAST_RESULT = None          # test.py introspection (exec_time_ns etc.)


def _pick_geometry(max_load):
    """Pick (cap, CH) minimizing modeled PE stream time."""
    best = None
    for ch in CH_OPTIONS:
        nchunk = max(1, -(-max_load // ch))
        cost = nchunk * (2 * KH * KD + KD * KH) * (ch / 2.4 + 2.5)
        if best is None or cost < best[0]:
            best = (cost, nchunk * ch, ch)
    return best[1], best[2]


def _build_ffn(cap, CH):
    """One-expert dense FFN, transposed layout. All cores run this program."""
    import concourse.bacc as bacc
    import concourse.mybir as mybir
    import concourse.tile as tile

    f32 = mybir.dt.float32
    f16 = mybir.dt.float16
    nchunk = cap // CH

    nc = bacc.Bacc(None, target_bir_lowering=False)
    xT = nc.dram_tensor("xT", [D, cap], f16, kind="ExternalInput")
    wgu = nc.dram_tensor("wgu", [D, 2 * H], f16, kind="ExternalInput")
    wd = nc.dram_tensor("wd", [H, D], f16, kind="ExternalInput")
    yT = nc.dram_tensor("yT", [D, cap], f32, kind="ExternalOutput")

    with tile.TileContext(nc) as tc:
        with (
            tc.tile_pool(name="wpool", bufs=1) as wpool,
            tc.tile_pool(name="xpool", bufs=2) as xpool,
            tc.tile_pool(name="apool", bufs=1) as apool,
            tc.tile_pool(name="spool", bufs=2) as spool,
            tc.tile_pool(name="ypool", bufs=3) as ypool,
            tc.tile_pool(name="psgu", bufs=6, space="PSUM") as psgu,
            tc.tile_pool(name="psy", bufs=2, space="PSUM") as psy,
        ):
            wgu_t = wgu.rearrange("(ko p) h -> p ko h", p=P)
            wd_t = wd.rearrange("(ko p) d -> p ko d", p=P)
            xT_t = xT.rearrange("(ko p) n -> p ko n", p=P)

            def load_x(c):
                # chunks >= 1: one DMA, prefetched under plenty of compute.
                # chunk 0 (prologue below) instead uses per-k DMAs so the
                # very first matmul only waits for x[k=0] + wgu[0].
                t = xpool.tile([P, KD, CH], f16, tag="x", name=f"xc{c}")
                nc.sync.dma_start(t[:], xT_t[:, :, c * CH:(c + 1) * CH])
                return [t[:, k, :] for k in range(KD)]

            x_sb = []
            wgu_sb = []
            for k in range(KD):
                tx = xpool.tile([P, CH], f16, tag=f"x0k{k}", name=f"x0k{k}")
                nc.sync.dma_start(tx[:], xT_t[:, k, 0:CH])
                x_sb.append(tx)
                tg = wpool.tile([P, 2 * H], f16, tag=f"wgu{k}", name=f"wgu{k}")
                nc.sync.dma_start(tg[:], wgu_t[:, k, :])
                wgu_sb.append(tg)
            wd_sb = []
            for k in range(KH):
                td = wpool.tile([P, D], f16, tag=f"wd{k}", name=f"wd{k}")
                nc.sync.dma_start(td[:], wd_t[:, k, :])
                wd_sb.append(td)

            for c in range(nchunk):
                if c > 0:
                    x_sb = load_x(c)
                a_sb = [
                    apool.tile([P, CH], f16, tag=f"a{h}", name=f"a{h}")
                    for h in range(KH)
                ]
                for h in range(KH):
                    g_ps = psgu.tile([P, CH], f32, tag="gu")
                    u_ps = psgu.tile([P, CH], f32, tag="gu")
                    gs = slice(h * P, (h + 1) * P)
                    us = slice(H + h * P, H + (h + 1) * P)
                    for k in range(KD):
                        nc.tensor.matmul(
                            g_ps[:], wgu_sb[k][:, gs], x_sb[k],
                            start=(k == 0), stop=(k == KD - 1),
                        )
                    for k in range(KD):
                        nc.tensor.matmul(
                            u_ps[:], wgu_sb[k][:, us], x_sb[k],
                            start=(k == 0), stop=(k == KD - 1),
                        )
                    s_sb = spool.tile([P, CH], f32, tag="s")
                    nc.scalar.activation(
                        s_sb[:], g_ps[:], mybir.ActivationFunctionType.Silu
                    )
                    nc.vector.tensor_mul(a_sb[h][:], s_sb[:], u_ps[:])

                for m in range(KD):
                    y_ps = psy.tile([P, CH], f32, tag="y")
                    ms = slice(m * P, (m + 1) * P)
                    for h in range(KH):
                        nc.tensor.matmul(
                            y_ps[:], wd_sb[h][:, ms], a_sb[h][:],
                            start=(h == 0), stop=(h == KH - 1),
                        )
                    y_sb = ypool.tile([P, CH], f32, tag="y")
                    nc.vector.tensor_copy(y_sb[:], y_ps[:])
                    nc.sync.dma_start(yT[ms, c * CH:(c + 1) * CH], y_sb[:])

    nc.compile()
    return nc


def _route(xf, Wr):
    """Top-2 routing + lb loss, bit-matching the reference (jax on CPU)."""
    import jax
    import jax.numpy as jnp

    cpu = jax.devices("cpu")[0]
    with jax.default_device(cpu):
        probs = jax.nn.softmax(jnp.asarray(xf) @ jnp.asarray(Wr), axis=-1)
        topk_p, topk_i = jax.lax.top_k(probs, TOPK)
        topk_p = topk_p / jnp.sum(topk_p, axis=-1, keepdims=True)
        usage = jnp.mean(probs, axis=0)
        lb = LB_WEIGHT * jnp.mean((usage - 1.0 / E) ** 2)
        return (
            np.asarray(topk_p),
            np.asarray(topk_i),
            np.asarray(lb, dtype=np.float32),
        )


def _run_spmd(nc, in_maps):
    global LAST_RESULT
    from concourse.bass_utils import run_bass_kernel_spmd

    res = run_bass_kernel_spmd(nc, in_maps, list(range(NCORES)))
    LAST_RESULT = res
    return res


def kernel(x, Wr, Wg, Wu, Wx, Wd, alpha):
    x = np.asarray(x, dtype=np.float32)
    Wr = np.asarray(Wr, dtype=np.float32)
    Wg = np.asarray(Wg, dtype=np.float32)
    Wu = np.asarray(Wu, dtype=np.float32)
    Wx = np.asarray(Wx, dtype=np.float32)
    Wd = np.asarray(Wd, dtype=np.float32)
    alpha = np.asarray(alpha, dtype=np.float32)

    xf = x.reshape(T, D)
    topk_p, topk_i, lb = _route(xf, Wr)

    # token ids + combine weight per expert
    ids = []
    wts = []
    for e in range(E):
        sel = (topk_i == e)
        tok = np.nonzero(sel.any(axis=1))[0]
        ids.append(tok)
        wts.append(np.where(sel[tok, 0], topk_p[tok, 0], topk_p[tok, 1]))
    cap, ch = _pick_geometry(max(len(t) for t in ids))

    if (cap, ch) not in _BUILD_CACHE:
        _BUILD_CACHE[(cap, ch)] = _build_ffn(cap, ch)
    nc = _BUILD_CACHE[(cap, ch)]

    xT = np.zeros((E, D, cap), dtype=np.float16)
    for e in range(E):
        xT[e, :, : len(ids[e])] = xf[ids[e]].T.astype(np.float16)

    in_maps = [
        {"xT": xT[e],
         "wgu": np.concatenate([Wg[e], Wu[e]], axis=1).astype(np.float16),
         "wd": Wd[e].astype(np.float16)}
        for e in range(E)
    ]
    res = _run_spmd(nc, in_maps)
    y = [res.results[e]["yT"] for e in range(E)]

    if not np.all(alpha == 1.0):
        in_maps2 = [
            dict(m, wgu=np.concatenate([Wx[e], Wu[e]], axis=1).astype(np.float16))
            for e, m in enumerate(in_maps)
        ]
        res2 = _run_spmd(nc, in_maps2)
        y = [alpha[e] * y[e] + (1.0 - alpha[e]) * res2.results[e]["yT"]
             for e in range(E)]

    out = np.zeros((T, D), dtype=np.float32)
    for e in range(E):
        n = len(ids[e])
        out[ids[e]] += wts[e][:, None] * y[e].T[:n]

    return out.reshape(B, S, D), lb


# revision 22
# speedup vs baseline: 1.0485x; 1.0268x over previous
"""MoE kernel for Trainium2 (8 NeuronCores, expert-parallel).

Strategy:
  - Router (tiny: [8192,1024]@[1024,8]) + top-2 selection + load-balance loss
    run on host with jax-CPU, bit-matching the reference implementation.
  - Tokens are gathered per expert on host ("all-to-all dispatch" done at
    input-sharding time) and shipped pre-transposed as xT=[D, cap] so the
    expert weights (Wgu [D,2H], Wd [H,D]) are used directly as matmul lhsT
    operands -- zero on-device transposes.
  - Each of the 8 cores runs the dense FFN for one expert in fp16 inputs /
    fp32 accumulate: G=x@Wg, U=x@Wu, A=silu(G)*U, Y=A@Wd.
  - Host applies the top-2 combine weights while scatter-adding each
    expert's rows back into the full [8192,1024] output ("combine").
  - alpha != 1 fallback: the same compiled kernel is run a second time with
    Wx in place of Wg, and the two outputs are blended per expert
    (alpha*Y1 + (1-alpha)*Y2) before the combine -- exact by linearity of
    the down-projection.
"""

import numpy as np

B, S, D = 4, 2048, 1024
E, TOPK, H = 8, 2, 1408
T = B * S
P = 128
CH_OPTIONS = (512, 448, 384, 320, 256)  # token chunk (matmul moving free dim)
KD, KH = D // P, H // P     # k-tiles over D, tiles over H
NCORES = 8
LB_WEIGHT = 0.01

_BUILD_CACHE = {}
LAST_RESULT = None          # test.py introspection (exec_time_ns etc.)


def _pick_chunks(max_load):
    """Balanced static chunk schedule: minimum chunk count at <=512 tokens
    per chunk, sizes multiples of 8, total padding <8 tokens."""
    nch = max(1, -(-max_load // 512))
    base = -(-max_load // nch)
    base = -(-base // 8) * 8
    chs = []
    left = max_load
    for _ in range(nch):
        n = min(base, max(8, -(-left // 8) * 8))
        chs.append(n)
        left -= n
    return tuple(chs)


def _build_ffn(chs):
    """One-expert dense FFN, transposed layout. All cores run this program.

    chs: static per-chunk token counts (sum = cap)."""
    import concourse.bacc as bacc
    import concourse.mybir as mybir
    import concourse.tile as tile

    f32 = mybir.dt.float32
    f16 = mybir.dt.float16
    cap = sum(chs)
    cmax = max(chs)

    nc = bacc.Bacc(None, target_bir_lowering=False)
    xT = nc.dram_tensor("xT", [D, cap], f16, kind="ExternalInput")
    wgu = nc.dram_tensor("wgu", [D, 2 * H], f16, kind="ExternalInput")
    wd = nc.dram_tensor("wd", [H, D], f16, kind="ExternalInput")
    yT = nc.dram_tensor("yT", [D, cap], f32, kind="ExternalOutput")

    with tile.TileContext(nc) as tc:
        with (
            tc.tile_pool(name="wpool", bufs=1) as wpool,
            tc.tile_pool(name="xpool", bufs=2) as xpool,
            tc.tile_pool(name="apool", bufs=1) as apool,
            tc.tile_pool(name="spool", bufs=2) as spool,
            tc.tile_pool(name="ypool", bufs=3) as ypool,
            tc.tile_pool(name="psgu", bufs=6, space="PSUM") as psgu,
            tc.tile_pool(name="psy", bufs=2, space="PSUM") as psy,
        ):
            wgu_t = wgu.rearrange("(ko p) h -> p ko h", p=P)
            wd_t = wd.rearrange("(ko p) d -> p ko d", p=P)
            xT_t = xT.rearrange("(ko p) n -> p ko n", p=P)

            def load_x(c, off, n):
                # chunks >= 1: one DMA, prefetched under plenty of compute.
                # chunk 0 (prologue below) instead uses per-k DMAs so the
                # very first matmul only waits for x[k=0] + wg[0].
                t = xpool.tile([P, KD, cmax], f16, tag="x", name=f"xc{c}")
                nc.sync.dma_start(t[:, :, :n], xT_t[:, :, off:off + n])
                return [t[:, k, :n] for k in range(KD)]

            # gate-half weights first: the first G accumulation group only
            # gates on wg (2.9MB), not the full fused gate+up stack
            x_sb = []
            wg_sb, wu_sb = [], []
            for k in range(KD):
                tx = xpool.tile([P, chs[0]], f16, tag=f"x0k{k}", name=f"x0k{k}")
                nc.sync.dma_start(tx[:], xT_t[:, k, 0:chs[0]])
                x_sb.append(tx[:])
                tg = wpool.tile([P, H], f16, tag=f"wg{k}", name=f"wg{k}")
                nc.sync.dma_start(tg[:], wgu_t[:, k, 0:H])
                wg_sb.append(tg)
            for k in range(KD):
                tu = wpool.tile([P, H], f16, tag=f"wu{k}", name=f"wu{k}")
                nc.sync.dma_start(tu[:], wgu_t[:, k, H:2 * H])
                wu_sb.append(tu)
            wd_sb = []
            for k in range(KH):
                td = wpool.tile([P, D], f16, tag=f"wd{k}", name=f"wd{k}")
                nc.sync.dma_start(td[:], wd_t[:, k, :])
                wd_sb.append(td)

            off = 0
            for c, n in enumerate(chs):
                if c > 0:
                    x_sb = load_x(c, off, n)
                a_sb = [
                    apool.tile([P, cmax], f16, tag=f"a{h}", name=f"a{h}")
                    for h in range(KH)
                ]
                for h in range(KH):
                    g_ps = psgu.tile([P, cmax], f32, tag="gu")
                    u_ps = psgu.tile([P, cmax], f32, tag="gu")
                    hs = slice(h * P, (h + 1) * P)
                    for k in range(KD):
                        nc.tensor.matmul(
                            g_ps[:, :n], wg_sb[k][:, hs], x_sb[k],
                            start=(k == 0), stop=(k == KD - 1),
                        )
                    for k in range(KD):
                        nc.tensor.matmul(
                            u_ps[:, :n], wu_sb[k][:, hs], x_sb[k],
                            start=(k == 0), stop=(k == KD - 1),
                        )
                    s_sb = spool.tile([P, cmax], f32, tag="s")
                    nc.scalar.activation(
                        s_sb[:, :n], g_ps[:, :n], mybir.ActivationFunctionType.Silu
                    )
                    nc.vector.tensor_mul(a_sb[h][:, :n], s_sb[:, :n], u_ps[:, :n])

                for m in range(KD):
                    y_ps = psy.tile([P, cmax], f32, tag="y")
                    ms = slice(m * P, (m + 1) * P)
                    for h in range(KH):
                        nc.tensor.matmul(
                            y_ps[:, :n], wd_sb[h][:, ms], a_sb[h][:, :n],
                            start=(h == 0), stop=(h == KH - 1),
                        )
                    y_sb = ypool.tile([P, cmax], f32, tag="y")
                    nc.vector.tensor_copy(y_sb[:, :n], y_ps[:, :n])
                    nc.sync.dma_start(yT[ms, off:off + n], y_sb[:, :n])
                off += n

    nc.compile()
    return nc


def _route(xf, Wr):
    """Top-2 routing + lb loss, bit-matching the reference (jax on CPU)."""
    import jax
    import jax.numpy as jnp

    cpu = jax.devices("cpu")[0]
    with jax.default_device(cpu):
        probs = jax.nn.softmax(jnp.asarray(xf) @ jnp.asarray(Wr), axis=-1)
        topk_p, topk_i = jax.lax.top_k(probs, TOPK)
        topk_p = topk_p / jnp.sum(topk_p, axis=-1, keepdims=True)
        usage = jnp.mean(probs, axis=0)
        lb = LB_WEIGHT * jnp.mean((usage - 1.0 / E) ** 2)
        return (
            np.asarray(topk_p),
            np.asarray(topk_i),
            np.asarray(lb, dtype=np.float32),
        )


def _run_spmd(nc, in_maps):
    global LAST_RESULT
    from concourse.bass_utils import run_bass_kernel_spmd

    res = run_bass_kernel_spmd(nc, in_maps, list(range(NCORES)))
    LAST_RESULT = res
    return res


def kernel(x, Wr, Wg, Wu, Wx, Wd, alpha):
    x = np.asarray(x, dtype=np.float32)
    Wr = np.asarray(Wr, dtype=np.float32)
    Wg = np.asarray(Wg, dtype=np.float32)
    Wu = np.asarray(Wu, dtype=np.float32)
    Wx = np.asarray(Wx, dtype=np.float32)
    Wd = np.asarray(Wd, dtype=np.float32)
    alpha = np.asarray(alpha, dtype=np.float32)

    xf = x.reshape(T, D)
    topk_p, topk_i, lb = _route(xf, Wr)

    # token ids + combine weight per expert
    ids = []
    wts = []
    for e in range(E):
        sel = (topk_i == e)
        tok = np.nonzero(sel.any(axis=1))[0]
        ids.append(tok)
        wts.append(np.where(sel[tok, 0], topk_p[tok, 0], topk_p[tok, 1]))
    chs = _pick_chunks(max(len(t) for t in ids))
    cap = sum(chs)

    if chs not in _BUILD_CACHE:
        _BUILD_CACHE[chs] = _build_ffn(chs)
    nc = _BUILD_CACHE[chs]

    xT = np.zeros((E, D, cap), dtype=np.float16)
    for e in range(E):
        xT[e, :, : len(ids[e])] = xf[ids[e]].T.astype(np.float16)

    in_maps = [
        {"xT": xT[e],
         "wgu": np.concatenate([Wg[e], Wu[e]], axis=1).astype(np.float16),
         "wd": Wd[e].astype(np.float16)}
        for e in range(E)
    ]
    res = _run_spmd(nc, in_maps)
    y = [res.results[e]["yT"] for e in range(E)]

    if not np.all(alpha == 1.0):
        in_maps2 = [
            dict(m, wgu=np.concatenate([Wx[e], Wu[e]], axis=1).astype(np.float16))
            for e, m in enumerate(in_maps)
        ]
        res2 = _run_spmd(nc, in_maps2)
        y = [alpha[e] * y[e] + (1.0 - alpha[e]) * res2.results[e]["yT"]
             for e in range(E)]

    out = np.zeros((T, D), dtype=np.float32)
    for e in range(E):
        n = len(ids[e])
        out[ids[e]] += wts[e][:, None] * y[e].T[:n]

    return out.reshape(B, S, D), lb


# revision 23
# speedup vs baseline: 1.0487x; 1.0002x over previous
"""MoE kernel for Trainium2 (8 NeuronCores, expert-parallel).

Strategy:
  - Router (tiny: [8192,1024]@[1024,8]) + top-2 selection + load-balance loss
    run on host with jax-CPU, bit-matching the reference implementation.
  - Tokens are gathered per expert on host ("all-to-all dispatch" done at
    input-sharding time) and shipped pre-transposed as xT=[D, cap] so the
    expert weights (Wgu [D,2H], Wd [H,D]) are used directly as matmul lhsT
    operands -- zero on-device transposes.
  - Each of the 8 cores runs the dense FFN for one expert in fp16 inputs /
    fp32 accumulate: G=x@Wg, U=x@Wu, A=silu(G)*U, Y=A@Wd.
  - Host applies the top-2 combine weights while scatter-adding each
    expert's rows back into the full [8192,1024] output ("combine").
  - alpha != 1 fallback: the same compiled kernel is run a second time with
    Wx in place of Wg, and the two outputs are blended per expert
    (alpha*Y1 + (1-alpha)*Y2) before the combine -- exact by linearity of
    the down-projection.
"""

import numpy as np

B, S, D = 4, 2048, 1024
E, TOPK, H = 8, 2, 1408
T = B * S
P = 128
CH_OPTIONS = (512, 448, 384, 320, 256)  # token chunk (matmul moving free dim)
KD, KH = D // P, H // P     # k-tiles over D, tiles over H
NCORES = 8
LB_WEIGHT = 0.01

_BUILD_CACHE = {}
LAST_RESULT = None          # test.py introspection (exec_time_ns etc.)


def _pick_chunks(max_load):
    """Balanced static chunk schedule: minimum chunk count at <=512 tokens
    per chunk, sizes multiples of 8, total padding <8 tokens."""
    nch = max(1, -(-max_load // 512))
    base = -(-max_load // nch)
    base = -(-base // 8) * 8
    chs = []
    left = max_load
    for _ in range(nch):
        n = min(base, max(8, -(-left // 8) * 8))
        chs.append(n)
        left -= n
    return tuple(chs)


def _build_ffn(chs):
    """One-expert dense FFN, transposed layout. All cores run this program.

    chs: static per-chunk token counts (sum = cap)."""
    import concourse.bacc as bacc
    import concourse.mybir as mybir
    import concourse.tile as tile

    f32 = mybir.dt.float32
    f16 = mybir.dt.float16
    cap = sum(chs)
    cmax = max(chs)

    nc = bacc.Bacc(None, target_bir_lowering=False)
    xT = nc.dram_tensor("xT", [D, cap], f16, kind="ExternalInput")
    wgu = nc.dram_tensor("wgu", [D, 2 * H], f16, kind="ExternalInput")
    wd = nc.dram_tensor("wd", [H, D], f16, kind="ExternalInput")
    yT = nc.dram_tensor("yT", [D, cap], f32, kind="ExternalOutput")

    with tile.TileContext(nc) as tc:
        with (
            tc.tile_pool(name="wpool", bufs=1) as wpool,
            tc.tile_pool(name="xpool", bufs=2) as xpool,
            tc.tile_pool(name="apool", bufs=1) as apool,
            tc.tile_pool(name="spool", bufs=2) as spool,
            tc.tile_pool(name="ypool", bufs=3) as ypool,
            tc.tile_pool(name="psgu", bufs=6, space="PSUM") as psgu,
            tc.tile_pool(name="psy", bufs=2, space="PSUM") as psy,
        ):
            wgu_t = wgu.rearrange("(ko p) h -> p ko h", p=P)
            wd_t = wd.rearrange("(ko p) d -> p ko d", p=P)
            xT_t = xT.rearrange("(ko p) n -> p ko n", p=P)

            def load_x(c, off, n):
                # chunks >= 1: one DMA, prefetched under plenty of compute.
                # chunk 0 (prologue below) instead uses per-k DMAs so the
                # very first matmul only waits for x[k=0] + wg[0].
                t = xpool.tile([P, KD, cmax], f16, tag="x", name=f"xc{c}")
                nc.sync.dma_start(t[:, :, :n], xT_t[:, :, off:off + n])
                return [t[:, k, :n] for k in range(KD)]

            # gate-half weights first: the first G accumulation group only
            # gates on wg (2.9MB), not the full fused gate+up stack
            x_sb = []
            wg_sb, wu_sb = [], []
            for k in range(KD):
                tx = xpool.tile([P, chs[0]], f16, tag=f"x0k{k}", name=f"x0k{k}")
                nc.sync.dma_start(tx[:], xT_t[:, k, 0:chs[0]])
                x_sb.append(tx[:])
                tg = wpool.tile([P, H], f16, tag=f"wg{k}", name=f"wg{k}")
                nc.sync.dma_start(tg[:], wgu_t[:, k, 0:H])
                wg_sb.append(tg)
            for k in range(KD):
                tu = wpool.tile([P, H], f16, tag=f"wu{k}", name=f"wu{k}")
                nc.sync.dma_start(tu[:], wgu_t[:, k, H:2 * H])
                wu_sb.append(tu)
            wd_sb = []
            for k in range(KH):
                td = wpool.tile([P, D], f16, tag=f"wd{k}", name=f"wd{k}")
                nc.sync.dma_start(td[:], wd_t[:, k, :])
                wd_sb.append(td)

            off = 0
            for c, n in enumerate(chs):
                if c > 0:
                    x_sb = load_x(c, off, n)
                a_sb = [
                    apool.tile([P, cmax], f16, tag=f"a{h}", name=f"a{h}")
                    for h in range(KH)
                ]
                # G runs LAG h-groups ahead of U so the PE has ready G-work
                # while the up-half weights are still arriving (chunk 0)
                LAG = 3
                g_live = {}
                for step in range(KH + LAG):
                    if step < KH:
                        g_ps = psgu.tile([P, cmax], f32, tag="gu")
                        hs = slice(step * P, (step + 1) * P)
                        for k in range(KD):
                            nc.tensor.matmul(
                                g_ps[:, :n], wg_sb[k][:, hs], x_sb[k],
                                start=(k == 0), stop=(k == KD - 1),
                            )
                        g_live[step] = g_ps
                    h = step - LAG
                    if h < 0:
                        continue
                    u_ps = psgu.tile([P, cmax], f32, tag="gu")
                    hs = slice(h * P, (h + 1) * P)
                    for k in range(KD):
                        nc.tensor.matmul(
                            u_ps[:, :n], wu_sb[k][:, hs], x_sb[k],
                            start=(k == 0), stop=(k == KD - 1),
                        )
                    g_ps = g_live.pop(h)
                    s_sb = spool.tile([P, cmax], f32, tag="s")
                    nc.scalar.activation(
                        s_sb[:, :n], g_ps[:, :n], mybir.ActivationFunctionType.Silu
                    )
                    nc.vector.tensor_mul(a_sb[h][:, :n], s_sb[:, :n], u_ps[:, :n])

                for m in range(KD):
                    y_ps = psy.tile([P, cmax], f32, tag="y")
                    ms = slice(m * P, (m + 1) * P)
                    for h in range(KH):
                        nc.tensor.matmul(
                            y_ps[:, :n], wd_sb[h][:, ms], a_sb[h][:, :n],
                            start=(h == 0), stop=(h == KH - 1),
                        )
                    y_sb = ypool.tile([P, cmax], f32, tag="y")
                    nc.vector.tensor_copy(y_sb[:, :n], y_ps[:, :n])
                    nc.sync.dma_start(yT[ms, off:off + n], y_sb[:, :n])
                off += n

    nc.compile()
    return nc


def _route(xf, Wr):
    """Top-2 routing + lb loss, bit-matching the reference (jax on CPU)."""
    import jax
    import jax.numpy as jnp

    cpu = jax.devices("cpu")[0]
    with jax.default_device(cpu):
        probs = jax.nn.softmax(jnp.asarray(xf) @ jnp.asarray(Wr), axis=-1)
        topk_p, topk_i = jax.lax.top_k(probs, TOPK)
        topk_p = topk_p / jnp.sum(topk_p, axis=-1, keepdims=True)
        usage = jnp.mean(probs, axis=0)
        lb = LB_WEIGHT * jnp.mean((usage - 1.0 / E) ** 2)
        return (
            np.asarray(topk_p),
            np.asarray(topk_i),
            np.asarray(lb, dtype=np.float32),
        )


def _run_spmd(nc, in_maps):
    global LAST_RESULT
    from concourse.bass_utils import run_bass_kernel_spmd

    res = run_bass_kernel_spmd(nc, in_maps, list(range(NCORES)))
    LAST_RESULT = res
    return res


def kernel(x, Wr, Wg, Wu, Wx, Wd, alpha):
    x = np.asarray(x, dtype=np.float32)
    Wr = np.asarray(Wr, dtype=np.float32)
    Wg = np.asarray(Wg, dtype=np.float32)
    Wu = np.asarray(Wu, dtype=np.float32)
    Wx = np.asarray(Wx, dtype=np.float32)
    Wd = np.asarray(Wd, dtype=np.float32)
    alpha = np.asarray(alpha, dtype=np.float32)

    xf = x.reshape(T, D)
    topk_p, topk_i, lb = _route(xf, Wr)

    # token ids + combine weight per expert
    ids = []
    wts = []
    for e in range(E):
        sel = (topk_i == e)
        tok = np.nonzero(sel.any(axis=1))[0]
        ids.append(tok)
        wts.append(np.where(sel[tok, 0], topk_p[tok, 0], topk_p[tok, 1]))
    chs = _pick_chunks(max(len(t) for t in ids))
    cap = sum(chs)

    if chs not in _BUILD_CACHE:
        _BUILD_CACHE[chs] = _build_ffn(chs)
    nc = _BUILD_CACHE[chs]

    xT = np.zeros((E, D, cap), dtype=np.float16)
    for e in range(E):
        xT[e, :, : len(ids[e])] = xf[ids[e]].T.astype(np.float16)

    in_maps = [
        {"xT": xT[e],
         "wgu": np.concatenate([Wg[e], Wu[e]], axis=1).astype(np.float16),
         "wd": Wd[e].astype(np.float16)}
        for e in range(E)
    ]
    res = _run_spmd(nc, in_maps)
    y = [res.results[e]["yT"] for e in range(E)]

    if not np.all(alpha == 1.0):
        in_maps2 = [
            dict(m, wgu=np.concatenate([Wx[e], Wu[e]], axis=1).astype(np.float16))
            for e, m in enumerate(in_maps)
        ]
        res2 = _run_spmd(nc, in_maps2)
        y = [alpha[e] * y[e] + (1.0 - alpha[e]) * res2.results[e]["yT"]
             for e in range(E)]

    out = np.zeros((T, D), dtype=np.float32)
    for e in range(E):
        n = len(ids[e])
        out[ids[e]] += wts[e][:, None] * y[e].T[:n]

    return out.reshape(B, S, D), lb


# revision 25
# speedup vs baseline: 1.0564x; 1.0074x over previous
"""MoE kernel for Trainium2 (8 NeuronCores, expert-parallel).

Strategy:
  - Router (tiny: [8192,1024]@[1024,8]) + top-2 selection + load-balance loss
    run on host with jax-CPU, bit-matching the reference implementation.
  - Tokens are gathered per expert on host ("all-to-all dispatch" done at
    input-sharding time) and shipped pre-transposed as xT=[D, cap] so the
    expert weights (Wgu [D,2H], Wd [H,D]) are used directly as matmul lhsT
    operands -- zero on-device transposes.
  - Each of the 8 cores runs the dense FFN for one expert in fp16 inputs /
    fp32 accumulate: G=x@Wg, U=x@Wu, A=silu(G)*U, Y=A@Wd.
  - Host applies the top-2 combine weights while scatter-adding each
    expert's rows back into the full [8192,1024] output ("combine").
  - alpha != 1 fallback: the same compiled kernel is run a second time with
    Wx in place of Wg, and the two outputs are blended per expert
    (alpha*Y1 + (1-alpha)*Y2) before the combine -- exact by linearity of
    the down-projection.
"""

import numpy as np

B, S, D = 4, 2048, 1024
E, TOPK, H = 8, 2, 1408
T = B * S
P = 128
CH_OPTIONS = (512, 448, 384, 320, 256)  # token chunk (matmul moving free dim)
KD, KH = D // P, H // P     # k-tiles over D, tiles over H
NCORES = 8
LB_WEIGHT = 0.01

_BUILD_CACHE = {}
LAST_RESULT = None          # test.py introspection (exec_time_ns etc.)


def _pick_chunks(max_load):
    """Balanced static chunk schedule: minimum chunk count at <=512 tokens
    per chunk, sizes multiples of 8, total padding <8 tokens."""
    nch = max(1, -(-max_load // 512))
    base = -(-max_load // nch)
    base = -(-base // 8) * 8
    chs = []
    left = max_load
    for _ in range(nch):
        n = min(base, max(8, -(-left // 8) * 8))
        chs.append(n)
        left -= n
    return tuple(chs)


def _build_ffn(chs):
    """One-expert dense FFN, transposed layout. All cores run this program.

    chs: static per-chunk token counts (sum = cap)."""
    import concourse.bacc as bacc
    import concourse.mybir as mybir
    import concourse.tile as tile

    f32 = mybir.dt.float32
    f16 = mybir.dt.float16
    cap = sum(chs)
    cmax = max(chs)

    nc = bacc.Bacc(None, target_bir_lowering=False)
    xT = nc.dram_tensor("xT", [D, cap], f16, kind="ExternalInput")
    wgu = nc.dram_tensor("wgu", [D, 2 * H], f16, kind="ExternalInput")
    wd = nc.dram_tensor("wd", [H, D], f16, kind="ExternalInput")
    yT = nc.dram_tensor("yT", [D, cap], f32, kind="ExternalOutput")

    with tile.TileContext(nc) as tc:
        with (
            tc.tile_pool(name="wpool", bufs=1) as wpool,
            tc.tile_pool(name="xpool", bufs=2) as xpool,
            tc.tile_pool(name="apool", bufs=1) as apool,
            tc.tile_pool(name="spool", bufs=2) as spool,
            tc.tile_pool(name="ypool", bufs=3) as ypool,
            tc.tile_pool(name="psgu", bufs=6, space="PSUM") as psgu,
            tc.tile_pool(name="psy", bufs=2, space="PSUM") as psy,
        ):
            wgu_t = wgu.rearrange("(ko p) h -> p ko h", p=P)
            wd_t = wd.rearrange("(ko p) d -> p ko d", p=P)
            xT_t = xT.rearrange("(ko p) n -> p ko n", p=P)

            def load_x(c, off, n):
                # chunks >= 1: one DMA, prefetched under plenty of compute.
                # chunk 0 (prologue below) instead uses per-k DMAs so the
                # very first matmul only waits for x[k=0] + wg[0].
                t = xpool.tile([P, KD, cmax], f16, tag="x", name=f"xc{c}")
                nc.sync.dma_start(t[:, :, :n], xT_t[:, :, off:off + n])
                return [t[:, k, :n] for k in range(KD)]

            # gate-half weights first: the first G accumulation group only
            # gates on wg (2.9MB), not the full fused gate+up stack
            x_sb = []
            wg_sb, wu_sb = [], []
            for k in range(KD):
                tx = xpool.tile([P, chs[0]], f16, tag=f"x0k{k}", name=f"x0k{k}")
                nc.sync.dma_start(tx[:], xT_t[:, k, 0:chs[0]])
                x_sb.append(tx[:])
                tg = wpool.tile([P, H], f16, tag=f"wg{k}", name=f"wg{k}")
                nc.sync.dma_start(tg[:], wgu_t[:, k, 0:H])
                wg_sb.append(tg)
            for k in range(KD):
                tu = wpool.tile([P, H], f16, tag=f"wu{k}", name=f"wu{k}")
                nc.sync.dma_start(tu[:], wgu_t[:, k, H:2 * H])
                wu_sb.append(tu)
            wd_sb = []
            for k in range(KH):
                td = wpool.tile([P, D], f16, tag=f"wd{k}", name=f"wd{k}")
                nc.sync.dma_start(td[:], wd_t[:, k, :])
                wd_sb.append(td)

            off = 0
            for c, n in enumerate(chs):
                if c > 0:
                    x_sb = load_x(c, off, n)
                a_sb = [
                    apool.tile([P, cmax], f16, tag=f"a{h}", name=f"a{h}")
                    for h in range(KH)
                ]
                # chunk 0: k-major over h-blocks of 3 so every arriving wg
                # k-slice unlocks 3 matmuls instead of 1 (weight-delivery
                # window); later chunks: plain h-major with G 3 ahead of U
                if c == 0:
                    for lo in range(0, KH, 3):
                        hb = range(lo, min(lo + 3, KH))
                        g_ps = {h: psgu.tile([P, cmax], f32, tag="gu", name=f"g{h}") for h in hb}
                        u_ps = {h: psgu.tile([P, cmax], f32, tag="gu", name=f"u{h}") for h in hb}
                        for k in range(KD):
                            for h in hb:
                                nc.tensor.matmul(
                                    g_ps[h][:, :n],
                                    wg_sb[k][:, h * P:(h + 1) * P], x_sb[k],
                                    start=(k == 0), stop=(k == KD - 1),
                                )
                        for k in range(KD):
                            for h in hb:
                                nc.tensor.matmul(
                                    u_ps[h][:, :n],
                                    wu_sb[k][:, h * P:(h + 1) * P], x_sb[k],
                                    start=(k == 0), stop=(k == KD - 1),
                                )
                        for h in hb:
                            s_sb = spool.tile([P, cmax], f32, tag="s")
                            nc.scalar.activation(
                                s_sb[:, :n], g_ps[h][:, :n],
                                mybir.ActivationFunctionType.Silu,
                            )
                            nc.vector.tensor_mul(
                                a_sb[h][:, :n], s_sb[:, :n], u_ps[h][:, :n]
                            )
                else:
                    LAG = 3
                    g_live = {}
                    for step in range(KH + LAG):
                        if step < KH:
                            g_ps = psgu.tile([P, cmax], f32, tag="gu")
                            hs = slice(step * P, (step + 1) * P)
                            for k in range(KD):
                                nc.tensor.matmul(
                                    g_ps[:, :n], wg_sb[k][:, hs], x_sb[k],
                                    start=(k == 0), stop=(k == KD - 1),
                                )
                            g_live[step] = g_ps
                        h = step - LAG
                        if h < 0:
                            continue
                        u_ps = psgu.tile([P, cmax], f32, tag="gu")
                        hs = slice(h * P, (h + 1) * P)
                        for k in range(KD):
                            nc.tensor.matmul(
                                u_ps[:, :n], wu_sb[k][:, hs], x_sb[k],
                                start=(k == 0), stop=(k == KD - 1),
                            )
                        g_ps = g_live.pop(h)
                        s_sb = spool.tile([P, cmax], f32, tag="s")
                        nc.scalar.activation(
                            s_sb[:, :n], g_ps[:, :n],
                            mybir.ActivationFunctionType.Silu,
                        )
                        nc.vector.tensor_mul(
                            a_sb[h][:, :n], s_sb[:, :n], u_ps[:, :n]
                        )

                for m in range(KD):
                    y_ps = psy.tile([P, cmax], f32, tag="y")
                    ms = slice(m * P, (m + 1) * P)
                    for h in range(KH):
                        nc.tensor.matmul(
                            y_ps[:, :n], wd_sb[h][:, ms], a_sb[h][:, :n],
                            start=(h == 0), stop=(h == KH - 1),
                        )
                    y_sb = ypool.tile([P, cmax], f32, tag="y")
                    nc.vector.tensor_copy(y_sb[:, :n], y_ps[:, :n])
                    nc.sync.dma_start(yT[ms, off:off + n], y_sb[:, :n])
                off += n

    nc.compile()
    return nc


def _route(xf, Wr):
    """Top-2 routing + lb loss, bit-matching the reference (jax on CPU)."""
    import jax
    import jax.numpy as jnp

    cpu = jax.devices("cpu")[0]
    with jax.default_device(cpu):
        probs = jax.nn.softmax(jnp.asarray(xf) @ jnp.asarray(Wr), axis=-1)
        topk_p, topk_i = jax.lax.top_k(probs, TOPK)
        topk_p = topk_p / jnp.sum(topk_p, axis=-1, keepdims=True)
        usage = jnp.mean(probs, axis=0)
        lb = LB_WEIGHT * jnp.mean((usage - 1.0 / E) ** 2)
        return (
            np.asarray(topk_p),
            np.asarray(topk_i),
            np.asarray(lb, dtype=np.float32),
        )


def _run_spmd(nc, in_maps):
    global LAST_RESULT
    from concourse.bass_utils import run_bass_kernel_spmd

    res = run_bass_kernel_spmd(nc, in_maps, list(range(NCORES)))
    LAST_RESULT = res
    return res


def kernel(x, Wr, Wg, Wu, Wx, Wd, alpha):
    x = np.asarray(x, dtype=np.float32)
    Wr = np.asarray(Wr, dtype=np.float32)
    Wg = np.asarray(Wg, dtype=np.float32)
    Wu = np.asarray(Wu, dtype=np.float32)
    Wx = np.asarray(Wx, dtype=np.float32)
    Wd = np.asarray(Wd, dtype=np.float32)
    alpha = np.asarray(alpha, dtype=np.float32)

    xf = x.reshape(T, D)
    topk_p, topk_i, lb = _route(xf, Wr)

    # token ids + combine weight per expert
    ids = []
    wts = []
    for e in range(E):
        sel = (topk_i == e)
        tok = np.nonzero(sel.any(axis=1))[0]
        ids.append(tok)
        wts.append(np.where(sel[tok, 0], topk_p[tok, 0], topk_p[tok, 1]))
    chs = _pick_chunks(max(len(t) for t in ids))
    cap = sum(chs)

    if chs not in _BUILD_CACHE:
        _BUILD_CACHE[chs] = _build_ffn(chs)
    nc = _BUILD_CACHE[chs]

    xT = np.zeros((E, D, cap), dtype=np.float16)
    for e in range(E):
        xT[e, :, : len(ids[e])] = xf[ids[e]].T.astype(np.float16)

    in_maps = [
        {"xT": xT[e],
         "wgu": np.concatenate([Wg[e], Wu[e]], axis=1).astype(np.float16),
         "wd": Wd[e].astype(np.float16)}
        for e in range(E)
    ]
    res = _run_spmd(nc, in_maps)
    y = [res.results[e]["yT"] for e in range(E)]

    if not np.all(alpha == 1.0):
        in_maps2 = [
            dict(m, wgu=np.concatenate([Wx[e], Wu[e]], axis=1).astype(np.float16))
            for e, m in enumerate(in_maps)
        ]
        res2 = _run_spmd(nc, in_maps2)
        y = [alpha[e] * y[e] + (1.0 - alpha[e]) * res2.results[e]["yT"]
             for e in range(E)]

    out = np.zeros((T, D), dtype=np.float32)
    for e in range(E):
        n = len(ids[e])
        out[ids[e]] += wts[e][:, None] * y[e].T[:n]

    return out.reshape(B, S, D), lb


# revision 27
# speedup vs baseline: 1.0625x; 1.0057x over previous
"""MoE kernel for Trainium2 (8 NeuronCores, expert-parallel).

Strategy:
  - Router (tiny: [8192,1024]@[1024,8]) + top-2 selection + load-balance loss
    run on host with jax-CPU, bit-matching the reference implementation.
  - Tokens are gathered per expert on host ("all-to-all dispatch" done at
    input-sharding time) and shipped pre-transposed as xT=[D, cap] so the
    expert weights (Wgu [D,2H], Wd [H,D]) are used directly as matmul lhsT
    operands -- zero on-device transposes.
  - Each of the 8 cores runs the dense FFN for one expert in fp16 inputs /
    fp32 accumulate: G=x@Wg, U=x@Wu, A=silu(G)*U, Y=A@Wd.
  - Host applies the top-2 combine weights while scatter-adding each
    expert's rows back into the full [8192,1024] output ("combine").
  - alpha != 1 fallback: the same compiled kernel is run a second time with
    Wx in place of Wg, and the two outputs are blended per expert
    (alpha*Y1 + (1-alpha)*Y2) before the combine -- exact by linearity of
    the down-projection.
"""

import numpy as np

B, S, D = 4, 2048, 1024
E, TOPK, H = 8, 2, 1408
T = B * S
P = 128
CH_OPTIONS = (512, 448, 384, 320, 256)  # token chunk (matmul moving free dim)
KD, KH = D // P, H // P     # k-tiles over D, tiles over H
NCORES = 8
LB_WEIGHT = 0.01

_BUILD_CACHE = {}
LAST_RESULT = None          # test.py introspection (exec_time_ns etc.)


def _pick_chunks(max_load):
    """Balanced static chunk schedule: minimum chunk count at <=512 tokens
    per chunk, sizes multiples of 8, total padding <8 tokens."""
    nch = max(1, -(-max_load // 512))
    base = -(-max_load // nch)
    base = -(-base // 8) * 8
    chs = []
    left = max_load
    for _ in range(nch):
        n = min(base, max(8, -(-left // 8) * 8))
        chs.append(n)
        left -= n
    return tuple(chs)


def _build_ffn(chs):
    """One-expert dense FFN, transposed layout. All cores run this program.

    chs: static per-chunk token counts (sum = cap)."""
    import concourse.bacc as bacc
    import concourse.mybir as mybir
    import concourse.tile as tile

    f32 = mybir.dt.float32
    f16 = mybir.dt.float16
    cap = sum(chs)
    cmax = max(chs)

    nc = bacc.Bacc(None, target_bir_lowering=False)
    xT = nc.dram_tensor("xT", [D, cap], f16, kind="ExternalInput")
    wgu = nc.dram_tensor("wgu", [D, 2 * H], f16, kind="ExternalInput")
    wd = nc.dram_tensor("wd", [H, D], f16, kind="ExternalInput")
    yT = nc.dram_tensor("yT", [D, cap], f32, kind="ExternalOutput")

    with tile.TileContext(nc) as tc:
        with (
            tc.tile_pool(name="wpool", bufs=1) as wpool,
            tc.tile_pool(name="xpool", bufs=2) as xpool,
            tc.tile_pool(name="apool", bufs=1) as apool,
            tc.tile_pool(name="spool", bufs=2) as spool,
            tc.tile_pool(name="ypool", bufs=3) as ypool,
            tc.tile_pool(name="psgu", bufs=6, space="PSUM") as psgu,
            tc.tile_pool(name="psy", bufs=2, space="PSUM") as psy,
        ):
            wgu_t = wgu.rearrange("(ko p) h -> p ko h", p=P)
            wd_t = wd.rearrange("(ko p) d -> p ko d", p=P)
            xT_t = xT.rearrange("(ko p) n -> p ko n", p=P)

            def load_x(c, off, n):
                # chunks >= 1: one DMA, prefetched under plenty of compute.
                # chunk 0 (prologue below) instead uses per-k DMAs so the
                # very first matmul only waits for x[k=0] + wg[0].
                t = xpool.tile([P, KD, cmax], f16, tag="x", name=f"xc{c}")
                nc.sync.dma_start(t[:, :, :n], xT_t[:, :, off:off + n])
                return [t[:, k, :n] for k in range(KD)]

            # gate-half weights first: the first G accumulation group only
            # gates on wg (2.9MB), not the full fused gate+up stack
            x_sb = []
            wg_sb, wu_sb = [], []
            for k in range(KD):
                tx = xpool.tile([P, chs[0]], f16, tag=f"x0k{k}", name=f"x0k{k}")
                nc.sync.dma_start(tx[:], xT_t[:, k, 0:chs[0]])
                x_sb.append(tx[:])
                tg = wpool.tile([P, H], f16, tag=f"wg{k}", name=f"wg{k}")
                nc.sync.dma_start(tg[:], wgu_t[:, k, 0:H])
                wg_sb.append(tg)
            for k in range(KD):
                tu = wpool.tile([P, H], f16, tag=f"wu{k}", name=f"wu{k}")
                nc.sync.dma_start(tu[:], wgu_t[:, k, H:2 * H])
                wu_sb.append(tu)
            wd_sb = []
            for k in range(KH):
                td = wpool.tile([P, D], f16, tag=f"wd{k}", name=f"wd{k}")
                nc.sync.dma_start(td[:], wd_t[:, k, :])
                wd_sb.append(td)

            off = 0
            for c, n in enumerate(chs):
                if c > 0:
                    x_sb = load_x(c, off, n)
                a_sb = [
                    apool.tile([P, cmax], f16, tag=f"a{h}", name=f"a{h}")
                    for h in range(KH)
                ]
                # chunk 0: k-major over h-blocks of 3 so every arriving wg
                # k-slice unlocks 3 matmuls instead of 1 (weight-delivery
                # window); later chunks: plain h-major with G 3 ahead of U
                if c == 0:
                    for lo in range(0, KH, 3):
                        hb = range(lo, min(lo + 3, KH))
                        g_ps = {h: psgu.tile([P, cmax], f32, tag="gu", name=f"g{h}") for h in hb}
                        u_ps = {h: psgu.tile([P, cmax], f32, tag="gu", name=f"u{h}") for h in hb}
                        for k in range(KD):
                            for h in hb:
                                nc.tensor.matmul(
                                    g_ps[h][:, :n],
                                    wg_sb[k][:, h * P:(h + 1) * P], x_sb[k],
                                    start=(k == 0), stop=(k == KD - 1),
                                )
                        for k in range(KD):
                            for h in hb:
                                nc.tensor.matmul(
                                    u_ps[h][:, :n],
                                    wu_sb[k][:, h * P:(h + 1) * P], x_sb[k],
                                    start=(k == 0), stop=(k == KD - 1),
                                )
                        for h in hb:
                            s_sb = spool.tile([P, cmax], f32, tag="s")
                            nc.scalar.activation(
                                s_sb[:, :n], g_ps[h][:, :n],
                                mybir.ActivationFunctionType.Silu,
                            )
                            nc.vector.tensor_mul(
                                a_sb[h][:, :n], s_sb[:, :n], u_ps[h][:, :n]
                            )
                else:
                    LAG = 3
                    g_live = {}
                    for step in range(KH + LAG):
                        if step < KH:
                            g_ps = psgu.tile([P, cmax], f32, tag="gu")
                            hs = slice(step * P, (step + 1) * P)
                            for k in range(KD):
                                nc.tensor.matmul(
                                    g_ps[:, :n], wg_sb[k][:, hs], x_sb[k],
                                    start=(k == 0), stop=(k == KD - 1),
                                )
                            g_live[step] = g_ps
                        h = step - LAG
                        if h < 0:
                            continue
                        u_ps = psgu.tile([P, cmax], f32, tag="gu")
                        hs = slice(h * P, (h + 1) * P)
                        for k in range(KD):
                            nc.tensor.matmul(
                                u_ps[:, :n], wu_sb[k][:, hs], x_sb[k],
                                start=(k == 0), stop=(k == KD - 1),
                            )
                        g_ps = g_live.pop(h)
                        s_sb = spool.tile([P, cmax], f32, tag="s")
                        nc.scalar.activation(
                            s_sb[:, :n], g_ps[:, :n],
                            mybir.ActivationFunctionType.Silu,
                        )
                        nc.vector.tensor_mul(
                            a_sb[h][:, :n], s_sb[:, :n], u_ps[:, :n]
                        )

                for m in range(KD):
                    y_ps = psy.tile([P, cmax], f32, tag="y")
                    ms = slice(m * P, (m + 1) * P)
                    for h in range(KH):
                        nc.tensor.matmul(
                            y_ps[:, :n], wd_sb[h][:, ms], a_sb[h][:, :n],
                            start=(h == 0), stop=(h == KH - 1),
                        )
                    y_sb = ypool.tile([P, cmax], f32, tag="y")
                    nc.vector.tensor_copy(y_sb[:, :n], y_ps[:, :n])
                    nc.sync.dma_start(yT[ms, off:off + n], y_sb[:, :n])
                off += n

    nc.compile()
    return nc


def _route(xf, Wr):
    """Top-2 routing + lb loss, bit-matching the reference (jax on CPU)."""
    import jax
    import jax.numpy as jnp

    cpu = jax.devices("cpu")[0]
    with jax.default_device(cpu):
        probs = jax.nn.softmax(jnp.asarray(xf) @ jnp.asarray(Wr), axis=-1)
        topk_p, topk_i = jax.lax.top_k(probs, TOPK)
        topk_p = topk_p / jnp.sum(topk_p, axis=-1, keepdims=True)
        usage = jnp.mean(probs, axis=0)
        lb = LB_WEIGHT * jnp.mean((usage - 1.0 / E) ** 2)
        return (
            np.asarray(topk_p),
            np.asarray(topk_i),
            np.asarray(lb, dtype=np.float32),
        )


def _run_spmd(nc, in_maps):
    global LAST_RESULT
    from concourse.bass_utils import run_bass_kernel_spmd

    res = run_bass_kernel_spmd(nc, in_maps, list(range(NCORES)))
    LAST_RESULT = res
    return res


def kernel(x, Wr, Wg, Wu, Wx, Wd, alpha):
    x = np.asarray(x, dtype=np.float32)
    Wr = np.asarray(Wr, dtype=np.float32)
    Wg = np.asarray(Wg, dtype=np.float32)
    Wu = np.asarray(Wu, dtype=np.float32)
    Wx = np.asarray(Wx, dtype=np.float32)
    Wd = np.asarray(Wd, dtype=np.float32)
    alpha = np.asarray(alpha, dtype=np.float32)

    xf = x.reshape(T, D)
    topk_p, topk_i, lb = _route(xf, Wr)

    # token ids + combine weight per expert
    ids = []
    wts = []
    for e in range(E):
        sel = (topk_i == e)
        tok = np.nonzero(sel.any(axis=1))[0]
        ids.append(tok)
        wts.append(np.where(sel[tok, 0], topk_p[tok, 0], topk_p[tok, 1]))
    chs = _pick_chunks(max(len(t) for t in ids))
    cap = sum(chs)

    if chs not in _BUILD_CACHE:
        _BUILD_CACHE[chs] = _build_ffn(chs)
    nc = _BUILD_CACHE[chs]

    xT = np.zeros((E, D, cap), dtype=np.float16)
    for e in range(E):
        xT[e, :, : len(ids[e])] = xf[ids[e]].T.astype(np.float16)

    in_maps = [
        {"xT": xT[e],
         "wgu": np.concatenate([Wg[e], Wu[e]], axis=1).astype(np.float16),
         "wd": Wd[e].astype(np.float16)}
        for e in range(E)
    ]
    res = _run_spmd(nc, in_maps)
    y = [res.results[e]["yT"] for e in range(E)]

    if not np.all(alpha == 1.0):
        in_maps2 = [
            dict(m, wgu=np.concatenate([Wx[e], Wu[e]], axis=1).astype(np.float16))
            for e, m in enumerate(in_maps)
        ]
        res2 = _run_spmd(nc, in_maps2)
        y = [alpha[e] * y[e] + (1.0 - alpha[e]) * res2.results[e]["yT"]
             for e in range(E)]

    out = np.zeros((T, D), dtype=np.float32)
    for e in range(E):
        n = len(ids[e])
        out[ids[e]] += wts[e][:, None] * y[e].T[:n]

    return out.reshape(B, S, D), lb
